# revision 3
# baseline (speedup 1.0000x reference)
"""GAT layer on trn2, v6: raw-bass, minimal-sync, engine-balanced design.

Cost model measured in this environment (per instruction, on-stream):
  PE matmul ~40us (size-independent), ACT ~30us, DVE ~22-52us,
  gpsimd tensor op ~8us, satisfied semaphore wait ~124us, sem update +29us.
So: minimize per-engine instruction counts (especially PE), use very coarse
sync (few waits), put elementwise work on gpsimd, broadcast rows via
DRAM-bounce DMAs, and do partition reductions with gpsimd C-reduce plus
weight-vector matmul tricks.

Math (per core, one batch element; S = 2^-SHIFT scale for fp16 safety):
  c1 = Ww^T a1, c2 = Ww^T a2, b1 = a1.W_b, b2 = a2.W_b          (host)
  Wh1_i = (h c1)_i + b1 ;  Wh2_j = (h c2)_j + b2
  rho_i = S*e^{0.8 Wh1_i} (bcast rib); sig_j = e^{Wh2_j}; rj_j = S*e^{0.2 Wh2_j}
  w[j,i] = adj[i,j] * max(rho_i*sig_j, rj_j)        ( = S * w_true )
  Z[f,i] = sum_j h[j,f] w[j,i]   (PE 64 mm)  -> Zsb = Z * 2^-4 fp16
  y_H[o,i] = sum_f Ww[o,f] Zsb[f,i]                  (PE 4 mm)
  s1H[i] = Sw.Zsb[:,i], sYb[i] = bW2.Zsb[:,i]        (PE 4 mm)
  D = colsum(sum_t w_t) (gp C-reduce);  Dn = D * 2^-4  (same scale as y)
  s1 = s1H + Sb*Dn ; s2 = s2H + Dn*(Sb2*Dn + sYb) ; u = s1/128
  var = s2/128 - u^2 ; e2 = eps*Dn^2 ; rs = 1/sqrt(var+e2)
  out[o,i] = leaky( y_H*rs_i + b_o*(Dn*rs)_i + (-u*rs)_i )
LayerNorm+softmax make the result invariant to the common scale of (y, Dn).
"""
import sys

sys.path.insert(0, "/opt/trn_rl_repo")

from contextlib import ExitStack

import numpy as np

import concourse.bass as bass
import concourse.mybir as mybir
from concourse.bass_utils import run_bass_kernel_spmd

dt = mybir.dt
A = mybir.ActivationFunctionType
Op = mybir.AluOpType

N = 2048
F = 128
T = N // 128          # 16
EPS = 1e-5
N_CORES = 8
LN2 = float(np.log(2.0))
SHIFT = 8             # w scaled by 2^-SHIFT via exp-bias shift
ZSC = 0.0625          # Z drain scale 2^-4

# ---- blob layout (int32 columns) ----
ADJ_C = 8192                      # adjT fp8e4 [q, t*2048+i]
HN_C = ADJ_C + 1024               # h_nat fp16 [q, t*128+f]
HT_C = HN_C + 1024                # hT fp16 [f, i]
C2B_C = HT_C + 1024               # c2 tiled fp16 [*, t*128+f]
WWT_C = C2B_C + 64                # WwT fp16 [f, o]
SV_C = WWT_C + 1                  # [Sw | bW2] fp16 [f, 2]
BCOL_C = SV_C + 1                 # W_b f32 [o, 1]
C1_C = BCOL_C + 1                 # c1 f32 [f, 1]
B2_C = C1_C + 1                   # 2*W_b f32 [o, 1]
SCAL_C = B2_C + 5                 # f32 scalars x5 (replicated rows)
BLOB_C = SCAL_C
SC_RIBB = 0   # 0.8*b1 - SHIFT*ln2
SC_SIGB = 1   # b2
SC_RJB = 2    # 0.2*b2 - SHIFT*ln2
SC_SB = 3     # sum(W_b)
SC_SB2 = 4    # sum(W_b^2)


def build_gat_nc(reps=1, internal=False, trunc=None):
    import os
    trunc = trunc or os.environ.get('V6_TRUNC') or None
    nc = bass.Bass()
    if internal:
        BLOB = nc.dram_tensor("blob", [128, BLOB_C], dt.int32, kind="Internal")
        DUM = nc.dram_tensor("dum", [1, 64], dt.int32, kind="ExternalInput")
    else:
        BLOB = nc.dram_tensor("blob", [128, BLOB_C], dt.int32,
                              kind="ExternalInput")
    OUT = nc.dram_tensor("out", [128, N], dt.float16, kind="ExternalOutput")
    RB1 = nc.dram_tensor("rb1", [N], dt.float16, kind="Internal")
    RB2 = nc.dram_tensor("rb2", [3 * N], dt.float16, kind="Internal")

    with ExitStack() as st:
        env = {}

        def sb(name, shape, dty):
            env[name] = st.enter_context(nc.sbuf_tensor("sb_" + name, shape, dty))
            return env[name]

        blob = sb("blob", [128, BLOB_C], dt.int32)
        rib = sb("rib", [128, N], dt.float16)
        sig = sb("sig", [128, T], dt.float32)
        rj = sb("rj", [128, T], dt.float32)
        w2c = sb("w2c", [128, T], dt.float32)
        wsb = sb("wsb", [128, T * N], dt.float16)
        mx = sb("mx", [128, N], dt.float16)
        mxG = sb("mxG", [128, N], dt.float16)
        wacc = sb("wacc", [128, N], dt.float32)
        zsb = sb("zsb", [128, N], dt.float16)
        ysb = sb("ysb", [128, N], dt.float32)
        srsb = sb("srsb", [1, N], dt.float32)
        rows = sb("rows", [1, 3 * N], dt.float32)
        eprow = sb("eprow", [1, 3 * N], dt.float16)
        epb = sb("epb", [128, 3 * N], dt.float16)
        outsb = sb("outsb", [128, N], dt.float16)
        psum = st.enter_context(nc.psum_tensor("ps", [128, 4096],
                                               dt.float32))

        # views
        bh = blob[:].bitcast(dt.float16)
        bf = blob[:].bitcast(dt.float32)
        b8 = blob[:].bitcast(dt.float8e4)
        V = dict(
            adjT=b8[:, 0:4 * ADJ_C],
            h_nat=bh[:, 2 * ADJ_C:2 * HN_C],
            hT=bh[:, 2 * HN_C:2 * HT_C],
            c2b=bh[:, 2 * HT_C:2 * C2B_C],
            WwT=bh[:, 2 * C2B_C:2 * WWT_C],
            sv=bh[:, 2 * WWT_C:2 * WWT_C + 2],
            bcol=bf[:, SV_C:SV_C + 1],
            c1col=bf[:, BCOL_C:BCOL_C + 1],
            b2col=bf[:, C1_C:C1_C + 1],
            scal=bf[:, B2_C:B2_C + 5],
            z_p=psum[:, N:2 * N],
            y_p=psum[:, 0:N],
            sr_p=psum[0:1, N:2 * N],
            d_r=rows[0:1, 0:N],
            s2_r=rows[0:1, N:2 * N],
            v_r=rows[0:1, 2 * N:3 * N],
            u_r=srsb[0:1, :],          # s1H slot, overwritten in place
            rs_e=eprow[0:1, 0:N],
            nm_e=eprow[0:1, N:2 * N],
            rd_e=eprow[0:1, 2 * N:3 * N],
            rs_b=epb[:, 0:N],
            nm_b=epb[:, N:2 * N],
            rd_b=epb[:, 2 * N:3 * N],
            rho_row=mx[0:1, 0:N],      # scratch row inside mx
        )
        env.update(V)
        env["psum"] = psum
        sems = {k: st.enter_context(nc.semaphore(name=k)) for k in
                ["sDB", "sPR", "sRB", "sR1", "sR2", "sWC", "sSG", "sW",
                 "sZF", "sZD", "sY", "sYD", "sV2", "sRS", "sRW", "sWR", "sGS", "sE1",
                 "sE2", "sFN", "sOD"]}
        env.update(sems)
        env.update(BLOB=BLOB, OUT=OUT, RB1=RB1, RB2=RB2)
        if internal:
            dumt = st.enter_context(nc.sbuf_tensor("dumt", [1, 64],
                                                   dt.int32))
            nc.sync.dma_start(out=dumt[:], in_=DUM[:]).then_inc(
                sems["sOD"], 16)

        for r in range(reps):
            _emit_rep(nc, r, env, trunc)

    return nc


def _emit_rep(nc, r, e, trunc=None):
    (blob, rib, sig, rj, w2c, wsb, mx, mxG, wacc, zsb, ysb, srsb, eprow,
     epb, outsb) = (
        e["blob"], e["rib"], e["sig"], e["rj"], e["w2c"], e["wsb"], e["mx"],
        e["mxG"], e["wacc"], e["zsb"], e["ysb"], e["srsb"], e["eprow"],
        e["epb"], e["outsb"])
    adjT, h_nat, hT, c2b, WwT, sv, bcol, c1col, b2col, scal = (
        e["adjT"], e["h_nat"], e["hT"], e["c2b"], e["WwT"], e["sv"],
        e["bcol"], e["c1col"], e["b2col"], e["scal"])
    z_p, y_p, sr_p = e["z_p"], e["y_p"], e["sr_p"]
    d_r, s2_r, v_r, u_r = e["d_r"], e["s2_r"], e["v_r"], e["u_r"]
    rs_e, nm_e, rd_e = e["rs_e"], e["nm_e"], e["rd_e"]
    rs_b, nm_b, rd_b = e["rs_b"], e["nm_b"], e["rd_b"]
    rho_row = e["rho_row"]
    (sDB, sPR, sRB, sR1, sR2, sWC, sSG, sW, sZF, sZD, sY, sYD, sV2, sRS,
     sRW, sWR, sGS, sE1, sE2, sFN, sOD) = (
        e["sDB"], e["sPR"], e["sRB"], e["sR1"], e["sR2"], e["sWC"], e["sSG"],
        e["sW"], e["sZF"], e["sZD"], e["sY"], e["sYD"], e["sV2"], e["sRS"],
        e["sRW"], e["sWR"], e["sGS"], e["sE1"], e["sE2"], e["sFN"],
        e["sOD"])
    BLOB, OUT, RB1, RB2 = e["BLOB"], e["OUT"], e["RB1"], e["RB2"]
    R1 = r + 1

    # ================= SP: input DMA =================
    nc.sync.dma_start(out=blob[:], in_=BLOB[:]).then_inc(sDB, 16)

    # ================= PE =================
    for t in range(T):
        if t == 0:
            nc.tensor.wait_ge(sW, 4 * r + 2)
        elif t == 8:
            nc.tensor.wait_ge(sW, 4 * r + 4)
        lhs = h_nat[:, t * 128:(t + 1) * 128]
        for c in range(4):
            mm = nc.tensor.matmul(
                out=z_p[:, c * 512:(c + 1) * 512], lhsT=lhs,
                rhs=wsb[:, t * N + c * 512:t * N + (c + 1) * 512],
                start=(t == 0), stop=(t == T - 1))
    mm.then_inc(sZF, 1)
    if trunc in ("z", "z2"):
        return _emit_trunc_z(nc, r, e, trunc)
    nc.tensor.wait_ge(sZD, R1)
    for c in range(4):
        nc.tensor.matmul(out=y_p[:, c * 512:(c + 1) * 512], lhsT=WwT,
                         rhs=zsb[:, c * 512:(c + 1) * 512],
                         start=True, stop=True)
    for c in range(4):
        mm = nc.tensor.matmul(out=sr_p[:, c * 512:(c + 1) * 512],
                              lhsT=sv[:, 0:1],
                              rhs=zsb[:, c * 512:(c + 1) * 512],
                              start=True, stop=True)
    mm.then_inc(sY, 1)

    # ================= ACT =================
    nc.scalar.wait_ge(sWC, R1)
    nc.scalar.activation(out=sig[:], in_=w2c[:], func=A.Exp, scale=1.0,
                         bias=scal[:, SC_SIGB:SC_SIGB + 1])
    nc.scalar.activation(out=rj[:], in_=w2c[:], func=A.Exp, scale=0.2,
                         bias=scal[:, SC_RJB:SC_RJB + 1])
    nc.scalar.wait_ge(sPR, R1)
    nc.scalar.activation(out=rho_row, in_=v_r, func=A.Exp, scale=0.8,
                         bias=scal[0:1, SC_RIBB:SC_RIBB + 1]).then_inc(sRB, 1)
    nc.scalar.wait_ge(sZF, R1)
    nc.scalar.activation(out=zsb[:], in_=e["z_p"], func=A.Identity,
                         scale=ZSC).then_inc(sZD, 1)
    nc.scalar.wait_ge(sY, R1)
    if r > 0:
        nc.scalar.wait_ge(sFN, r)   # prev rep's final read of ysb done
    nc.scalar.activation(out=ysb[:], in_=y_p, func=A.Identity)
    nc.scalar.activation(out=u_r, in_=sr_p,
                         func=A.Identity).then_inc(sYD, 1)
    nc.scalar.wait_ge(sV2, R1)
    nc.scalar.activation(out=v_r, in_=v_r, func=A.Ln)
    nc.scalar.activation(out=rs_e, in_=v_r, func=A.Exp,
                         scale=-0.5).then_inc(sRS, 1)

    # ================= DVE =================
    nc.vector.wait_ge(sDB, 16 * R1)
    nc.vector.tensor_tensor(out=ysb[:], in0=h_nat, in1=c2b, op=Op.mult)
    scr3 = ysb[:].rearrange("p (t f) -> p t f", t=T)
    nc.vector.tensor_reduce(out=w2c[:], in_=scr3, axis=mybir.AxisListType.X,
                            op=Op.add).then_inc(sWC, 1)
    # even w tiles
    if r > 0:
        nc.vector.wait_ge(sZF, r)
    nc.vector.wait_ge(sR2, 16 * R1)
    for t in range(0, T, 2):
        nc.vector.tensor_scalar(out=mx[:], in0=rib[:],
                                scalar1=sig[:, t:t + 1],
                                scalar2=rj[:, t:t + 1],
                                op0=Op.mult, op1=Op.max)
        ti = nc.vector.tensor_tensor(out=wsb[:, t * N:(t + 1) * N],
                                     in0=mx[:],
                                     in1=adjT[:, t * N:(t + 1) * N],
                                     op=Op.mult)
        if t in (6, 14):
            ti.then_inc(sW, 1)
    nc.vector.wait_ge(sW, 4 * r + 4)
    wv = wsb[:].rearrange("p (t i) -> p i t", t=T)
    nc.vector.tensor_reduce(out=wacc[:], in_=wv, axis=mybir.AxisListType.X,
                            op=Op.add).then_inc(sWR, 1)
    # epilogue rows
    sc0 = scal[0:1, :]
    nc.vector.wait_ge(sGS, R1)
    # s2 = s2H + Dn*(Sb2*Dn + sYb)
    nc.vector.scalar_tensor_tensor(out=v_r, in0=d_r,
                                   scalar=sc0[:, SC_SB2:SC_SB2 + 1],
                                   in1=v_r, op0=Op.mult, op1=Op.add)
    nc.vector.tensor_tensor(out=v_r, in0=v_r, in1=d_r, op=Op.mult)
    nc.vector.tensor_tensor(out=s2_r, in0=s2_r, in1=v_r, op=Op.add)
    # u = (s1H + Sb*Dn)/128
    nc.vector.scalar_tensor_tensor(out=u_r, in0=d_r,
                                   scalar=sc0[:, SC_SB:SC_SB + 1],
                                   in1=u_r, op0=Op.mult, op1=Op.add)
    nc.vector.tensor_scalar(out=u_r, in0=u_r, scalar1=1.0 / 128,
                            scalar2=None, op0=Op.mult)
    # v2 = s2/128 - u^2 + eps*Dn^2
    nc.vector.tensor_tensor(out=v_r, in0=u_r, in1=u_r, op=Op.mult)
    nc.vector.scalar_tensor_tensor(out=v_r, in0=s2_r, scalar=1.0 / 128,
                                   in1=v_r, op0=Op.mult, op1=Op.subtract)
    nc.vector.scalar_tensor_tensor(out=s2_r, in0=d_r, scalar=EPS, in1=d_r,
                                   op0=Op.mult, op1=Op.mult)
    nc.vector.tensor_tensor(out=v_r, in0=v_r, in1=s2_r,
                            op=Op.add).then_inc(sV2, 1)
    nc.vector.wait_ge(sRS, R1)
    nc.vector.scalar_tensor_tensor(out=nm_e, in0=u_r, scalar=-1.0, in1=rs_e,
                                   op0=Op.mult, op1=Op.mult)
    nc.vector.tensor_tensor(out=rd_e, in0=d_r, in1=rs_e,
                            op=Op.mult).then_inc(sRW, 1)
    nc.vector.wait_ge(sE2, 16 * R1)
    nc.vector.tensor_tensor(out=ysb[:], in0=ysb[:], in1=rs_b, op=Op.mult)
    nc.vector.tensor_tensor(out=ysb[:], in0=ysb[:], in1=nm_b, op=Op.add)
    nc.vector.scalar_tensor_tensor(out=ysb[:], in0=rd_b, scalar=bcol,
                                   in1=ysb[:], op0=Op.mult, op1=Op.add)
    nc.vector.scalar_tensor_tensor(out=outsb[:], in0=ysb[:], scalar=0.2,
                                   in1=ysb[:], op0=Op.mult,
                                   op1=Op.max).then_inc(sFN, 1)

    # ================= GP =================
    nc.gpsimd.wait_ge(sDB, 16 * R1)
    # Wh1 row: (hT * c1) C-reduced over f -> v_r
    nc.gpsimd.tensor_scalar(out=wacc[:], in0=hT, scalar1=c1col, scalar2=None,
                            op0=Op.mult)
    nc.gpsimd.tensor_reduce(out=v_r, in_=wacc[:], axis=mybir.AxisListType.C,
                            op=Op.add).then_inc(sPR, 1)
    if r > 0:
        nc.gpsimd.wait_ge(sZF, r)       # don't clobber prev rep's w tiles
    nc.gpsimd.wait_ge(sR2, 16 * R1)     # rib bcast landed (implies sig/rj)
    for t in range(1, T, 2):
        nc.gpsimd.tensor_scalar(out=mxG[:], in0=rib[:],
                                scalar1=sig[:, t:t + 1],
                                scalar2=rj[:, t:t + 1],
                                op0=Op.mult, op1=Op.max)
        ti = nc.gpsimd.tensor_tensor(out=wsb[:, t * N:(t + 1) * N],
                                     in0=mxG[:],
                                     in1=adjT[:, t * N:(t + 1) * N],
                                     op=Op.mult)
        if t in (7, 15):
            ti.then_inc(sW, 1)
    # D row from wacc (DVE-produced t-sum)
    nc.gpsimd.wait_ge(sWR, R1)
    nc.gpsimd.tensor_reduce(out=d_r, in_=wacc[:], axis=mybir.AxisListType.C,
                            op=Op.add)
    nc.gpsimd.tensor_scalar(out=d_r, in0=d_r, scalar1=ZSC, scalar2=None,
                            op0=Op.mult)
    nc.gpsimd.wait_ge(sYD, R1)
    # sYb = colsum(2*b*y_H) -> v_r
    nc.gpsimd.tensor_scalar(out=wacc[:], in0=ysb[:], scalar1=b2col,
                            scalar2=None, op0=Op.mult)
    nc.gpsimd.tensor_reduce(out=v_r, in_=wacc[:], axis=mybir.AxisListType.C,
                            op=Op.add)
    # s2H = colsum(y_H^2) -> s2_r
    nc.gpsimd.tensor_tensor(out=wacc[:], in0=ysb[:], in1=ysb[:], op=Op.mult)
    nc.gpsimd.tensor_reduce(out=s2_r, in_=wacc[:], axis=mybir.AxisListType.C,
                            op=Op.add).then_inc(sGS, 1)

    # ================= SP: bounce DMAs + out =================
    nc.sync.wait_ge(sRB, R1)
    nc.sync.dma_start(out=RB1[:], in_=rho_row).then_inc(sR1, 16)
    nc.sync.wait_ge(sR1, 16 * R1)
    nc.sync.dma_start(
        out=rib[:],
        in_=RB1[:].unsqueeze(0).partition_broadcast(128)).then_inc(sR2, 16)
    nc.sync.wait_ge(sRW, R1)
    nc.sync.dma_start(out=RB2[:], in_=eprow[0:1, :]).then_inc(sE1, 16)
    nc.sync.wait_ge(sE1, 16 * R1)
    nc.sync.dma_start(
        out=epb[:],
        in_=RB2[:].unsqueeze(0).partition_broadcast(128)).then_inc(sE2, 16)
    nc.sync.wait_ge(sFN, R1)
    nc.sync.dma_start(out=OUT[:], in_=outsb[:]).then_inc(sOD, 16)


def _emit_trunc_z(nc, r, e, trunc='z'):
    """Truncated body: everything up to Z accumulation + drain, then out.
    For stage-timing only (output is garbage)."""
    R1 = r + 1
    (sDB, sRB, sR1, sR2, sWC, sSG, sW, sZF, sZD, sPR) = (
        e["sDB"], e["sRB"], e["sR1"], e["sR2"], e["sWC"], e["sSG"], e["sW"],
        e["sZF"], e["sZD"], e["sPR"])
    scal, rib, sig, rj, w2c, wsb, mx, mxG, wacc, zsb, ysb, h_nat, hT, c2b, \
        adjT, c1col = (
        e["scal"], e["rib"], e["sig"], e["rj"], e["w2c"], e["wsb"], e["mx"],
        e["mxG"], e["wacc"], e["zsb"], e["ysb"], e["h_nat"], e["hT"],
        e["c2b"], e["adjT"], e["c1col"])
    rho_row = e["rho_row"]
    BLOB, OUT, RB1 = e["BLOB"], e["OUT"], e["RB1"]
    N_, T_ = N, T
    Op_ = Op
    # ACT
    nc.scalar.wait_ge(sPR, R1)
    nc.scalar.activation(out=rho_row, in_=e["v_r"], func=A.Exp, scale=0.8,
                         bias=scal[0:1, SC_RIBB:SC_RIBB + 1]).then_inc(sRB, 1)
    nc.scalar.wait_ge(sWC, R1)
    nc.scalar.activation(out=sig[:], in_=w2c[:], func=A.Exp, scale=1.0,
                         bias=scal[:, SC_SIGB:SC_SIGB + 1])
    nc.scalar.activation(out=rj[:], in_=w2c[:], func=A.Exp, scale=0.2,
                         bias=scal[:, SC_RJB:SC_RJB + 1]).then_inc(sSG, 1)
    nc.scalar.wait_ge(sZF, R1)
    nc.scalar.activation(out=zsb[:], in_=e["z_p"], func=A.Identity,
                         scale=ZSC).then_inc(sZD, 1)
    # DVE
    nc.vector.wait_ge(sDB, 16 * R1)
    nc.vector.tensor_tensor(out=ysb[:], in0=h_nat, in1=c2b, op=Op_.mult)
    scr3 = ysb[:].rearrange("p (t f) -> p t f", t=T_)
    nc.vector.tensor_reduce(out=w2c[:], in_=scr3, axis=mybir.AxisListType.X,
                            op=Op_.add).then_inc(sWC, 1)
    if r > 0:
        nc.vector.wait_ge(sZF, r)
    nc.vector.wait_ge(sR2, 16 * R1)
    nc.vector.wait_ge(sSG, R1)
    for t in range(0, T_, 2):
        nc.vector.tensor_scalar(out=mx[:], in0=rib[:],
                                scalar1=sig[:, t:t + 1],
                                scalar2=rj[:, t:t + 1],
                                op0=Op_.mult, op1=Op_.max)
        ti = nc.vector.tensor_tensor(out=wsb[:, t * N_:(t + 1) * N_],
                                     in0=mx[:],
                                     in1=adjT[:, t * N_:(t + 1) * N_],
                                     op=Op_.mult)
        if t in (6, 14):
            ti.then_inc(sW, 1)
    if trunc != "z2":
        nc.vector.wait_ge(sW, 4 * r + 4)
        wv = wsb[:].rearrange("p (t i) -> p i t", t=T_)
        nc.vector.tensor_reduce(out=wacc[:], in_=wv,
                                axis=mybir.AxisListType.X,
                                op=Op_.add).then_inc(e["sWR"], 1)
    # GP
    nc.gpsimd.wait_ge(sDB, 16 * R1)
    nc.gpsimd.tensor_scalar(out=wacc[:], in0=hT, scalar1=c1col, scalar2=None,
                            op0=Op_.mult)
    nc.gpsimd.tensor_reduce(out=e["v_r"], in_=wacc[:],
                            axis=mybir.AxisListType.C,
                            op=Op_.add).then_inc(sPR, 1)
    if r > 0:
        nc.gpsimd.wait_ge(sZF, r)
    nc.gpsimd.wait_ge(sR2, 16 * R1)
    nc.gpsimd.wait_ge(sSG, R1)
    for t in range(1, T_, 2):
        nc.gpsimd.tensor_scalar(out=mxG[:], in0=rib[:],
                                scalar1=sig[:, t:t + 1],
                                scalar2=rj[:, t:t + 1],
                                op0=Op_.mult, op1=Op_.max)
        ti = nc.gpsimd.tensor_tensor(out=wsb[:, t * N_:(t + 1) * N_],
                                     in0=mxG[:],
                                     in1=adjT[:, t * N_:(t + 1) * N_],
                                     op=Op_.mult)
        if t in (7, 15):
            ti.then_inc(sW, 1)
    # SP
    nc.sync.wait_ge(sRB, R1)
    nc.sync.dma_start(out=RB1[:], in_=rho_row).then_inc(sR1, 16)
    nc.sync.wait_ge(sR1, 16 * R1)
    nc.sync.dma_start(
        out=rib[:],
        in_=RB1[:].unsqueeze(0).partition_broadcast(128)).then_inc(sR2, 16)
    nc.sync.wait_ge(sZD, R1)
    nc.sync.dma_start(out=OUT[:], in_=zsb[:]).then_inc(e["sOD"], 16)


# ---------------- host side ----------------

def pack_blobs(h, adj, W_w, W_b, a_w):
    B = h.shape[0]
    fp8 = np.dtype(mybir.dt.np(dt.float8e4))
    blob = np.zeros((B, 128, BLOB_C), np.int32)

    a1, a2 = a_w[:F], a_w[F:]
    c1 = W_w.T @ a1
    c2 = W_w.T @ a2
    b1 = float(a1 @ W_b)
    b2 = float(a2 @ W_b)
    Sw = W_w.sum(0)
    bW2 = 2.0 * (W_b @ W_w)
    Sb = float(W_b.sum())
    Sb2 = float((W_b ** 2).sum())

    def pack16(x16):
        x16 = np.ascontiguousarray(x16.astype(np.float16))
        if x16.shape[1] % 2:
            x16 = np.concatenate([x16, np.zeros((128, 1), np.float16)], 1)
        return x16.view(np.int32)

    for b in range(B):
        adjT = adj[b].T.astype(fp8)
        a8 = adjT.reshape(T, 128, N).transpose(1, 0, 2).reshape(128, T * N)
        blob[b, :, 0:ADJ_C] = np.ascontiguousarray(a8).view(np.int32)
        hn = h[b].reshape(T, 128, F).transpose(1, 0, 2).reshape(128, T * F)
        blob[b, :, ADJ_C:HN_C] = pack16(hn)
        blob[b, :, HN_C:HT_C] = pack16(np.ascontiguousarray(h[b].T))

    c2t = np.tile(c2.astype(np.float16)[None, :], (128, T))
    blob[:, :, HT_C:C2B_C] = pack16(c2t)[None]
    blob[:, :, C2B_C:WWT_C] = pack16(np.ascontiguousarray(W_w.T))[None]
    svol = np.stack([Sw, bW2], 1).astype(np.float16)
    blob[:, :, WWT_C:SV_C] = pack16(svol)[None]
    blob[:, :, SV_C:BCOL_C] = (
        W_b.astype(np.float32).reshape(128, 1).view(np.int32)[None])
    blob[:, :, BCOL_C:C1_C] = (
        c1.astype(np.float32).reshape(128, 1).view(np.int32)[None])
    blob[:, :, C1_C:B2_C] = (
        (2.0 * W_b).astype(np.float32).reshape(128, 1).view(np.int32)[None])
    scal = np.zeros((128, 5), np.float32)
    scal[:, SC_RIBB] = 0.8 * b1 - SHIFT * LN2
    scal[:, SC_SIGB] = b2
    scal[:, SC_RJB] = 0.2 * b2 - SHIFT * LN2
    scal[:, SC_SB] = Sb
    scal[:, SC_SB2] = Sb2
    blob[:, :, B2_C:SCAL_C] = scal.view(np.int32)[None]
    return blob


_NC_CACHE = None


def _get_nc():
    global _NC_CACHE
    if _NC_CACHE is None:
        _NC_CACHE = build_gat_nc()
    return _NC_CACHE


def kernel(h, adj, W_w, W_b, a_w):
    h = np.ascontiguousarray(np.asarray(h, dtype=np.float32))
    adj = np.ascontiguousarray(np.asarray(adj, dtype=np.int32))
    W_w = np.ascontiguousarray(np.asarray(W_w, dtype=np.float32))
    W_b = np.ascontiguousarray(np.asarray(W_b, dtype=np.float32)).reshape(F)
    a_w = np.ascontiguousarray(np.asarray(a_w, dtype=np.float32)).reshape(2 * F)

    B = h.shape[0]
    assert B == N_CORES and h.shape == (B, N, F) and adj.shape == (B, N, N)

    blob = pack_blobs(h, adj, W_w, W_b, a_w)
    nc = _get_nc()
    in_maps = [{"blob": blob[b]} for b in range(B)]
    res = run_bass_kernel_spmd(nc, in_maps, core_ids=list(range(N_CORES)))
    return np.stack([
        res.results[b]["out"].T.astype(np.float32) for b in range(B)
    ], axis=0)
